# revision 20
# baseline (speedup 1.0000x reference)
"""LSTM kernel for Trainium2 (Bass/Tile), 8-core data-parallel.

Model (per reference):
    xg = einsum('bsd,dg->sbg', x, Wi)            # input projections
    per step: z = xg_t + h @ Wh + bh
              i,f,g,o = split(z); c = sig(f)*c + sig(i)*tanh(g); h = sig(o)*tanh(c)
    out = h_last @ Wo + bo
Sharding: batch 256 -> 32 per core, weights replicated.

On-chip layout (per core):
  - gates-on-partitions: z for one step is a PSUM region [128, 128] laid out as
    [i|f|o|g] x 32 batch columns. Partition p = hidden feature; so i,f,o,g,c,h
    are all [H=128, B=32] tiles and h is directly the next matmul's rhs.
  - xg is precomputed by PE matmuls (lhsT = [Wi; bh] with a ones-row appended to
    x) straight into PSUM chunks of 16 steps; the per-step recurrence matmuls
    accumulate on top with start=False.
  - all matmul operands are bf16 (1 cycle/row on PE vs 4 for fp32); PSUM fp32.
  - the g-gate weights are pre-scaled by 2 on the host so tanh(zg) =
    2*sigmoid(2*zg) - 1 comes out of the SAME sigmoid activation as i,f,o
    (one ACT instr over all 128 gate columns); the affine fixup folds into
    the DVE ops:  u = (g' - 0.5)*i ;  c = 2*u + f*c  (scalar_tensor_tensor).
"""

import copy

import numpy as np

import concourse.bass as bass
import concourse.mybir as mybir
from concourse import tile
from concourse.bass_utils import run_bass_kernel_spmd

F32 = mybir.dt.float32
BF16 = mybir.dt.bfloat16
NP_BF16 = mybir.dt.np(mybir.dt.bfloat16)

B, S, D, H = 256, 4096, 64, 128
G4 = 4 * H  # 512
NCORES = 8
BC = B // NCORES  # 32 batch per core
TC = 16  # timesteps per PSUM chunk (4 banks)
BODY_CH = 4  # chunks per loop body (static x-slot / psum ping-pong)
KD = D + 1  # contraction rows for input projection (ones row folds bh in)
CPC = TC * BC  # x columns per chunk (512)
NG = 2  # independent batch groups interleaved to hide per-step chain latency
GB = BC // NG  # batch per group

# on-chip gate block order [i, f, o, g]; reference order is [i, f, g, o]
_PERM = np.concatenate(
    [np.arange(0, 128), np.arange(128, 256), np.arange(384, 512), np.arange(256, 384)]
)


def _legalize_for_walrus(nc):
    """Make the Tile-scheduled module lowerable by this walrus build.

    (1) This walrus accepts only ONE semaphore wait per TPB instruction
        (e.g. Matmult/LDWEIGHTS and DMACopy structs have a single wait slot);
        Tile emits multi-wait instructions. Hoist excess waits onto standalone
        EventSemaphore sequencer instructions placed just before, on the same
        engine — semantically identical (the sequencer blocks in order).
    (2) Drop the trailing EVENT_SEMAPHORE_RANGE_CLEAR InstISA (sem-recycling
        hygiene) which this walrus cannot lower at all.
    """
    f = nc.m.functions[0]
    template = None
    for blk in f.blocks:
        for inst in blk.instructions:
            if type(inst).__name__ == "InstEventSemaphore":
                template = inst
                break
        if template is not None:
            break
    assert template is not None, "no EventSemaphore to clone"
    uid = 0
    for blk in f.blocks:
        out = []
        for inst in blk.instructions:
            nm = type(inst).__name__
            if nm == "InstISA":
                continue  # (2)
            si = inst.sync_info
            waits = list(si.on_wait) if si is not None else []
            if nm != "InstEventSemaphore" and len(waits) > 1:
                for w in waits[1:]:
                    es = copy.deepcopy(template)
                    es.name = f"{inst.name}_hoist{uid}"
                    uid += 1
                    es.engine = inst.engine
                    es.sync_info = mybir.SyncInfo(on_wait=[w], on_update=[])
                    out.append(es)
                inst.sync_info = mybir.SyncInfo(
                    on_wait=waits[:1], on_update=list(si.on_update)
                )
            out.append(inst)
        blk.instructions = out


def build_bass(n_steps=S, legalize=True):
    n_ch = n_steps // TC
    assert n_ch % BODY_CH == 0 and n_steps % TC == 0
    n_iter = n_ch // BODY_CH
    pad_ch = n_ch + BODY_CH
    xcols = pad_ch * CPC

    nc = bass.Bass()
    xt = nc.declare_dram_parameter("xt", [KD, xcols], BF16, isOutput=False)
    # combined weights: cols [0:512] = Wh (permuted), cols [512:1024] = [Wi; bh]
    # (rows 65:128 of the right half are zero padding); g blocks pre-scaled x2
    wcb = nc.declare_dram_parameter("wcb", [H, 2 * G4], BF16, isOutput=False)
    hout = nc.declare_dram_parameter("h_out", [H, BC], F32, isOutput=True)

    with tile.TileContext(nc) as tc:
        with (
            tc.tile_pool(name="weights", bufs=1) as wpool,
            tc.tile_pool(name="xin", bufs=1) as xpool,
            tc.tile_pool(name="state", bufs=1) as spool,
            tc.tile_pool(name="psum", bufs=1, space=bass.MemorySpace.PSUM) as ppool,
        ):
            w_sb = wpool.tile([H, 2 * G4], BF16, tag="w")
            wh_sb = w_sb[:, 0:G4]
            wi_sb = w_sb[:KD, G4 : 2 * G4]
            xs_all = xpool.tile([KD, BODY_CH * CPC], BF16, tag="xs")
            xs = [xs_all[:, k * CPC : (k + 1) * CPC] for k in range(BODY_CH)]
            # per-group persistent state: st = sigmoid outputs [i|f|o|g'] (bf16),
            # cc = cell state (fp32), wk = [u|v] (fp32), tc_sb = tanh(c) (bf16)
            st = [
                spool.tile([H, 4 * GB], BF16, tag=f"st{g}", name=f"st{g}")
                for g in range(NG)
            ]
            cc = [
                spool.tile([H, GB], F32, tag=f"cc{g}", name=f"cc{g}")
                for g in range(NG)
            ]
            wk = [
                spool.tile([H, 2 * GB], F32, tag=f"wk{g}", name=f"wk{g}")
                for g in range(NG)
            ]
            tc_sb = [
                spool.tile([H, GB], BF16, tag=f"tc{g}", name=f"tc{g}")
                for g in range(NG)
            ]
            h_sb = [
                spool.tile([H, GB], BF16, tag=f"h{g}", name=f"h{g}")
                for g in range(NG)
            ]
            hf_sb = spool.tile([H, BC], F32, tag="hf")
            # one PSUM tile (2 banks) per (group, ping-pong slot): the Tile
            # dep-tracker works at tile granularity, so separate tiles keep
            # group A's sigmoid from waiting on group B's matmuls.
            ps = [
                [
                    ppool.tile(
                        [H, TC * 128 // NG], F32, tag=f"ps{g}_{k}", name=f"ps{g}_{k}"
                    )
                    for k in range(2)
                ]
                for g in range(NG)
            ]

            # chunk layout per psum tile: group g owns banks [2g, 2g+1]; within
            # its bank pair: [bank k (t//8)][gate block gb (4)][t%8][b (16)].
            # Groups touching disjoint banks keeps the Tile range-tracker from
            # fabricating cross-group deps (which would serialize the two
            # chains), and each start=True bank-zero stays inside one group.
            # x chunk columns are host-ordered [g (2)][t (16)][b (16)].
            def xg_items(pi, xsrc):
                """The 16 input-projection matmuls for one chunk into psum slot
                pi, as thunks (bank-zeroing gb==0 first per bank)."""
                items = []
                for g in range(NG):
                    for k in range(2):
                        for gb in range(4):

                            def mm(g=g, k=k, gb=gb):
                                nc.tensor.matmul(
                                    ps[g][pi][
                                        :,
                                        k * 512 + gb * 128 : k * 512 + (gb + 1) * 128,
                                    ],
                                    wi_sb[:, gb * H : (gb + 1) * H],
                                    xsrc[
                                        :, g * 256 + k * 128 : g * 256 + (k + 1) * 128
                                    ],
                                    start=(gb == 0),
                                    stop=False,
                                    skip_group_check=True,
                                )

                            items.append(mm)
                return items

            def xg_chunk(pi, xsrc):
                for it in xg_items(pi, xsrc):
                    it()

            def step(pi, j):
                """One LSTM timestep; z for step j=4q+r is strided inside bank q.

                The NG batch groups are independent recurrences; their per-step
                chains (PE -> sig -> u,c -> tanh -> h -> PE) are interleaved so
                group B's matmuls/sigmoid run while group A is in its DVE/tanh
                stage. Per-engine program order below is chosen so no engine
                stalls behind a not-yet-ready instruction of the other group.
                """
                k, tl = j // 8, j % 8  # bank within group's tile, step within bank
                act = mybir.ActivationFunctionType
                alu = mybir.AluOpType
                for g in range(NG):
                    zoff = k * 512 + tl * GB
                    for gb in range(4):
                        nc.tensor.matmul(
                            ps[g][pi][:, zoff + gb * 128 : zoff + gb * 128 + GB],
                            wh_sb[:, gb * H : (gb + 1) * H],
                            h_sb[g][:, :],
                            start=False,
                            stop=True,
                            skip_group_check=True,
                        )
                for g in range(NG):
                    # strided view: [bank (2)][gate block (4)][t%8][b (16)]
                    pz5 = ps[g][pi][:].rearrange(
                        "p (q gb t z) -> p q gb t z", q=2, gb=4, t=8
                    )
                    # one sigmoid over [i|f|o|g'] (g weights pre-scaled x2 on host)
                    nc.scalar.activation(
                        st[g][:].rearrange("p (a z) -> p a z", z=GB)[:, :, :],
                        pz5[:, k, :, tl, :],
                        act.Sigmoid,
                    )
                for g in range(NG):
                    # v = f * c  (reads previous c)
                    nc.vector.tensor_mul(
                        wk[g][:, GB : 2 * GB], st[g][:, GB : 2 * GB], cc[g][:, :]
                    )
                    # u = (g' - 0.5) * i   [= 0.5 * i * tanh(zg)]
                    nc.vector.scalar_tensor_tensor(
                        wk[g][:, 0:GB], st[g][:, 3 * GB : 4 * GB], 0.5, st[g][:, 0:GB],
                        alu.subtract, alu.mult,
                    )
                    # c = 2*u + v
                    nc.vector.scalar_tensor_tensor(
                        cc[g][:, :], wk[g][:, 0:GB], 2.0, wk[g][:, GB : 2 * GB],
                        alu.mult, alu.add,
                    )
                for g in range(NG):
                    nc.scalar.activation(tc_sb[g][:, :], cc[g][:, :], act.Tanh)
                for g in range(NG):
                    # h = o * tanh(c)
                    nc.vector.tensor_mul(
                        h_sb[g][:, :], st[g][:, 2 * GB : 3 * GB], tc_sb[g][:, :]
                    )

            def rec_chunk(pi, xg=()):
                """16 recurrence steps on slot pi; one xg matmul for the OTHER
                psum slot's next chunk is slipped in per step so the PE queue
                never sees a blocking burst of 16 xg matmuls at a chunk edge
                (the xg targets the other tile, so no WAR with this chunk's
                reads; its tile-level WAR on the previous chunk's sigmoid
                reads also keeps the bank-zeroing correctly ordered)."""
                for j in range(TC):
                    step(pi, j)
                    if j < len(xg):
                        xg[j]()

            # ---- preamble ----
            nc.sync.dma_start(w_sb[:], wcb[:])
            for g in range(NG):
                nc.vector.memset(h_sb[g][:], 0.0)
                nc.vector.memset(cc[g][:], 0.0)
            nc.sync.dma_start(xs_all[:], xt[:, 0 : BODY_CH * CPC])
            xg_chunk(0, xs[0])

            # ---- main loop: body covers chunks 4i .. 4i+3; rec of chunk n
            # carries the xg matmuls of chunk n+1 (which lives on the other
            # psum slot) ----
            with tc.For_i(
                0, n_iter, 1, hint_engines=(mybir.EngineType.PE,)
            ) as iv:
                base = iv * (BODY_CH * CPC)

                rec_chunk(0, xg_items(1, xs[1]))  # chunk 4i    (+xg 4i+1)
                rec_chunk(1, xg_items(0, xs[2]))  # chunk 4i+1  (+xg 4i+2)
                # split refill: each half lands >= 1 chunk before its consumer
                # so interleaved xg matmuls never hold the PE queue on a DMA
                nc.sync.dma_start(
                    xs_all[:, 0 : 2 * CPC],
                    xt[:, bass.ds(base + BODY_CH * CPC, 2 * CPC)],
                )
                rec_chunk(0, xg_items(1, xs[3]))  # chunk 4i+2  (+xg 4i+3)
                nc.sync.dma_start(
                    xs_all[:, 2 * CPC : 4 * CPC],
                    xt[:, bass.ds(base + (BODY_CH + 2) * CPC, 2 * CPC)],
                )
                rec_chunk(1, xg_items(0, xs[0]))  # chunk 4i+3  (+xg 4i+4)

            # widen the final h to fp32 for the output DMA
            for g in range(NG):
                nc.vector.tensor_scalar_add(
                    hf_sb[:, g * GB : (g + 1) * GB], h_sb[g][:, :], 0.0
                )
            nc.sync.dma_start(hout[:], hf_sb[:])

    if legalize:  # CoreSim can't run the post-hoc clones; HW compile needs them
        _legalize_for_walrus(nc)
    return nc


def host_inputs(x, Wi, Wh, bh, n_steps=S):
    """Per-core input maps: transposed/padded x (bf16), permuted weights (bf16,
    g block pre-scaled by 2 for the tanh-via-sigmoid trick)."""
    n_ch = n_steps // TC
    pad_ch = n_ch + BODY_CH
    xcols = pad_ch * CPC
    gscale = np.ones((G4,), np.float32)
    gscale[384:512] = 2.0  # post-perm cols 384:512 are the g block
    wcb = np.zeros((H, 2 * G4), np.float32)
    wcb[:, 0:G4] = Wh[:, _PERM] * gscale
    wcb[0:D, G4:] = Wi[:, _PERM] * gscale
    wcb[D, G4:] = bh[_PERM] * gscale
    wcb = wcb.astype(NP_BF16)
    nb = x.shape[0] // NCORES
    in_maps = []
    for core in range(NCORES):
        xc = x[core * nb : (core + 1) * nb]  # [BC, n_steps, D]
        xtc = np.ascontiguousarray(xc.transpose(2, 1, 0))  # [D, n_steps, BC]
        # within each 16-step chunk, reorder columns to [group][t][b]
        xtc = (
            xtc.reshape(D, n_ch, TC, NG, GB)
            .transpose(0, 1, 3, 2, 4)
            .reshape(D, n_steps * nb)
        )
        full = np.zeros((KD, xcols), NP_BF16)
        full[:D, : n_steps * nb] = xtc.astype(NP_BF16)
        full[D, :] = 1.0
        in_maps.append({"xt": full, "wcb": wcb})
    return in_maps


_CACHE = {}


def _run(x, Wi, Wh, bh, trace=False):
    x = np.asarray(x, np.float32)
    if "nc" not in _CACHE:
        _CACHE["nc"] = build_bass()
    nc = _CACHE["nc"]
    in_maps = host_inputs(x, Wi, Wh, bh)
    res = run_bass_kernel_spmd(nc, in_maps, list(range(NCORES)), trace=trace)
    h_full = np.concatenate(
        [np.asarray(res.results[c]["h_out"]).astype(np.float32).T for c in range(NCORES)],
        axis=0,
    )  # [B, H]
    return h_full, res


def kernel(x, Wi, Wh, bh, Wo, bo):
    x = np.asarray(x, np.float32)
    Wi = np.asarray(Wi, np.float32)
    Wh = np.asarray(Wh, np.float32)
    bh = np.asarray(bh, np.float32)
    Wo = np.asarray(Wo, np.float32)
    bo = np.asarray(bo, np.float32)
    h_full, _ = _run(x, Wi, Wh, bh)
    return (h_full @ Wo + bo).astype(np.float32)


# revision 21
# speedup vs baseline: 8.2305x; 8.2305x over previous
"""LSTM kernel for Trainium2 (Bass/Tile), 8-core data-parallel.

Model (per reference):
    xg = einsum('bsd,dg->sbg', x, Wi)            # input projections
    per step: z = xg_t + h @ Wh + bh
              i,f,g,o = split(z); c = sig(f)*c + sig(i)*tanh(g); h = sig(o)*tanh(c)
    out = h_last @ Wo + bo
Sharding: batch 256 -> 32 per core, weights replicated.

On-chip layout (per core):
  - gates-on-partitions: partition p = hidden feature; i,f,o,g,c,h are all
    [H=128, B=32] tiles and h is directly the next matmul's rhs.
  - PSUM: per 16-step chunk slot (2 slots ping-pong), PSUM bank gb holds gate
    block gb for the whole chunk: cols gb*512 + t*32 + b. One xg matmul per
    gate block covers the entire bank (N=512, start=True zeroes it); the
    per-step recurrence matmuls accumulate 32-col slices on top (start=False).
  - all matmul operands are bf16 (1 cycle/row on PE vs 4 for fp32); PSUM fp32.
  - the g-gate weights are pre-scaled by 2 on the host so tanh(zg) =
    2*sigmoid(2*zg) - 1 comes out of the SAME sigmoid activation as i,f,o
    (one ACT instr over all 4 gate blocks); the affine fixup folds into the
    DVE ops:  u = (g' - 0.5)*i ;  c = 2*u + f*c  (scalar_tensor_tensor).
  - xg matmuls for chunk n+1 are slipped one-per-4-steps into chunk n's
    recurrence so the in-order PE queue never stalls on an xg burst.
"""

import copy

import numpy as np

import concourse.bass as bass
import concourse.mybir as mybir
from concourse import tile
from concourse.bass_utils import run_bass_kernel_spmd

F32 = mybir.dt.float32
BF16 = mybir.dt.bfloat16
NP_BF16 = mybir.dt.np(mybir.dt.bfloat16)

B, S, D, H = 256, 4096, 64, 128
G4 = 4 * H  # 512
NCORES = 8
BC = B // NCORES  # 32 batch per core
TC = 16  # timesteps per PSUM chunk (4 banks)
BODY_CH = 4  # chunks per loop body (static x-slot / psum ping-pong)
KD = D + 1  # contraction rows for input projection (ones row folds bh in)
CPC = TC * BC  # x columns per chunk (512)

# on-chip gate block order [i, f, o, g]; reference order is [i, f, g, o]
_PERM = np.concatenate(
    [np.arange(0, 128), np.arange(128, 256), np.arange(384, 512), np.arange(256, 384)]
)


def _legalize_for_walrus(nc):
    """Make the Tile-scheduled module lowerable by this walrus build.

    (1) This walrus accepts only ONE semaphore wait per TPB instruction
        (e.g. Matmult/LDWEIGHTS and DMACopy structs have a single wait slot);
        Tile emits multi-wait instructions. Hoist excess waits onto standalone
        EventSemaphore sequencer instructions placed just before, on the same
        engine — semantically identical (the sequencer blocks in order).
    (2) Drop the trailing EVENT_SEMAPHORE_RANGE_CLEAR InstISA (sem-recycling
        hygiene) which this walrus cannot lower at all.
    """
    f = nc.m.functions[0]
    template = None
    for blk in f.blocks:
        for inst in blk.instructions:
            if type(inst).__name__ == "InstEventSemaphore":
                template = inst
                break
        if template is not None:
            break
    assert template is not None, "no EventSemaphore to clone"
    uid = 0
    for blk in f.blocks:
        out = []
        for inst in blk.instructions:
            nm = type(inst).__name__
            if nm == "InstISA":
                continue  # (2)
            si = inst.sync_info
            waits = list(si.on_wait) if si is not None else []
            if nm != "InstEventSemaphore" and len(waits) > 1:
                for w in waits[1:]:
                    es = copy.deepcopy(template)
                    es.name = f"{inst.name}_hoist{uid}"
                    uid += 1
                    es.engine = inst.engine
                    es.sync_info = mybir.SyncInfo(on_wait=[w], on_update=[])
                    out.append(es)
                inst.sync_info = mybir.SyncInfo(
                    on_wait=waits[:1], on_update=list(si.on_update)
                )
            out.append(inst)
        blk.instructions = out


def build_bass(n_steps=S, legalize=True):
    n_ch = n_steps // TC
    assert n_ch % BODY_CH == 0 and n_steps % TC == 0
    n_iter = n_ch // BODY_CH
    pad_ch = n_ch + BODY_CH
    xcols = pad_ch * CPC

    nc = bass.Bass()
    xt = nc.declare_dram_parameter("xt", [KD, xcols], BF16, isOutput=False)
    # combined weights: cols [0:512] = Wh (permuted), cols [512:1024] = [Wi; bh]
    # (rows 65:128 of the right half are zero padding); g blocks pre-scaled x2
    wcb = nc.declare_dram_parameter("wcb", [H, 2 * G4], BF16, isOutput=False)
    hout = nc.declare_dram_parameter("h_out", [H, BC], F32, isOutput=True)

    with tile.TileContext(nc) as tc:
        with (
            tc.tile_pool(name="weights", bufs=1) as wpool,
            tc.tile_pool(name="xin", bufs=1) as xpool,
            tc.tile_pool(name="state", bufs=1) as spool,
            tc.tile_pool(name="psum", bufs=1, space=bass.MemorySpace.PSUM) as ppool,
        ):
            w_sb = wpool.tile([H, 2 * G4], BF16, tag="w")
            wh_sb = w_sb[:, 0:G4]
            wi_sb = w_sb[:KD, G4 : 2 * G4]
            xs_all = xpool.tile([KD, BODY_CH * CPC], BF16, tag="xs")
            xs = [xs_all[:, k * CPC : (k + 1) * CPC] for k in range(BODY_CH)]
            # persistent state: st = sigmoid outputs [i|f|o|g'] (bf16),
            # cc = cell state (fp32), wk = [u|v] (fp32), tc_sb = tanh(c) (bf16)
            st = spool.tile([H, 4 * BC], BF16, tag="st")
            cc = spool.tile([H, BC], F32, tag="cc")
            wk = spool.tile([H, 2 * BC], F32, tag="wk")
            tc_sb = spool.tile([H, BC], BF16, tag="tc")
            h_sb = spool.tile([H, BC], BF16, tag="h")
            hf_sb = spool.tile([H, BC], F32, tag="hf")
            ps = [
                ppool.tile([H, TC * 128], F32, tag=f"ps{k}", name=f"ps{k}")
                for k in range(2)
            ]

            def xg_items(pi, xsrc):
                """The 4 input-projection matmuls for one chunk into psum slot
                pi (each covers one full bank = one gate block, start=True
                zeroes it), as thunks."""
                items = []
                for gb in range(4):

                    def mm(gb=gb):
                        nc.tensor.matmul(
                            ps[pi][:, gb * 512 : (gb + 1) * 512],
                            wi_sb[:, gb * H : (gb + 1) * H],
                            xsrc[:, :],
                            start=True,
                            stop=False,
                            skip_group_check=True,
                        )

                    items.append(mm)
                return items

            def xg_chunk(pi, xsrc):
                for it in xg_items(pi, xsrc):
                    it()

            def step(pi, j):
                """One LSTM timestep; z for step j lives at cols j*32 of each
                gate bank."""
                p = ps[pi]
                act = mybir.ActivationFunctionType
                alu = mybir.AluOpType
                for gb in range(4):
                    nc.tensor.matmul(
                        p[:, gb * 512 + j * BC : gb * 512 + (j + 1) * BC],
                        wh_sb[:, gb * H : (gb + 1) * H],
                        h_sb[:, :],
                        start=False,
                        stop=True,
                        skip_group_check=True,
                    )
                # strided view: the four gate blocks for step j sit 512 apart
                pz = p[:].rearrange("p (gb t z) -> p gb t z", gb=4, t=TC)
                # one sigmoid over [i|f|o|g'] (g weights pre-scaled x2 on host)
                nc.scalar.activation(
                    st[:].rearrange("p (a z) -> p a z", z=BC)[:, :, :],
                    pz[:, :, j, :],
                    act.Sigmoid,
                )
                # v = f * c  (reads previous c)
                nc.vector.tensor_mul(wk[:, BC : 2 * BC], st[:, BC : 2 * BC], cc[:, :])
                # u = (g' - 0.5) * i   [= 0.5 * i * tanh(zg)]
                nc.vector.scalar_tensor_tensor(
                    wk[:, 0:BC], st[:, 3 * BC : 4 * BC], 0.5, st[:, 0:BC],
                    alu.subtract, alu.mult,
                )
                # c = 2*u + v
                nc.vector.scalar_tensor_tensor(
                    cc[:, :], wk[:, 0:BC], 2.0, wk[:, BC : 2 * BC],
                    alu.mult, alu.add,
                )
                nc.scalar.activation(tc_sb[:, :], cc[:, :], act.Tanh)
                # h = o * tanh(c)
                nc.vector.tensor_mul(h_sb[:, :], st[:, 2 * BC : 3 * BC], tc_sb[:, :])

            def rec_chunk(pi, xg=()):
                """16 recurrence steps on slot pi; one xg matmul for the OTHER
                psum slot's next chunk is slipped in per 4 steps so the PE
                queue never sees a blocking burst at a chunk edge (tile-level
                WAR on the previous chunk's sigmoid reads keeps the bank
                zeroing correctly ordered)."""
                for j in range(TC):
                    step(pi, j)
                    if j % 4 == 3 and j // 4 < len(xg):
                        xg[j // 4]()

            # ---- preamble ----
            nc.sync.dma_start(w_sb[:], wcb[:])
            nc.vector.memset(h_sb[:], 0.0)
            nc.vector.memset(cc[:], 0.0)
            nc.sync.dma_start(xs_all[:], xt[:, 0 : BODY_CH * CPC])
            xg_chunk(0, xs[0])

            # ---- main loop: body covers chunks 4i .. 4i+3; rec of chunk n
            # carries the xg matmuls of chunk n+1 (on the other psum slot) ----
            with tc.For_i(
                0, n_iter, 1, hint_engines=(mybir.EngineType.PE,)
            ) as iv:
                base = iv * (BODY_CH * CPC)

                rec_chunk(0, xg_items(1, xs[1]))  # chunk 4i    (+xg 4i+1)
                rec_chunk(1, xg_items(0, xs[2]))  # chunk 4i+1  (+xg 4i+2)
                # split refill: each half lands >= 1 chunk before its consumer
                # so interleaved xg matmuls never hold the PE queue on a DMA
                nc.sync.dma_start(
                    xs_all[:, 0 : 2 * CPC],
                    xt[:, bass.ds(base + BODY_CH * CPC, 2 * CPC)],
                )
                rec_chunk(0, xg_items(1, xs[3]))  # chunk 4i+2  (+xg 4i+3)
                nc.sync.dma_start(
                    xs_all[:, 2 * CPC : 4 * CPC],
                    xt[:, bass.ds(base + (BODY_CH + 2) * CPC, 2 * CPC)],
                )
                rec_chunk(1, xg_items(0, xs[0]))  # chunk 4i+3  (+xg 4i+4)

            # widen the final h to fp32 for the output DMA
            nc.vector.tensor_scalar_add(hf_sb[:, :], h_sb[:, :], 0.0)
            nc.sync.dma_start(hout[:], hf_sb[:])

    if legalize:  # CoreSim can't run the post-hoc clones; HW compile needs them
        _legalize_for_walrus(nc)
    return nc


def host_inputs(x, Wi, Wh, bh, n_steps=S):
    """Per-core input maps: transposed/padded x (bf16), permuted weights (bf16,
    g block pre-scaled by 2 for the tanh-via-sigmoid trick)."""
    n_ch = n_steps // TC
    pad_ch = n_ch + BODY_CH
    xcols = pad_ch * CPC
    gscale = np.ones((G4,), np.float32)
    gscale[384:512] = 2.0  # post-perm cols 384:512 are the g block
    wcb = np.zeros((H, 2 * G4), np.float32)
    wcb[:, 0:G4] = Wh[:, _PERM] * gscale
    wcb[0:D, G4:] = Wi[:, _PERM] * gscale
    wcb[D, G4:] = bh[_PERM] * gscale
    wcb = wcb.astype(NP_BF16)
    nb = x.shape[0] // NCORES
    in_maps = []
    for core in range(NCORES):
        xc = x[core * nb : (core + 1) * nb]  # [BC, n_steps, D]
        xtc = np.ascontiguousarray(xc.transpose(2, 1, 0)).reshape(D, n_steps * nb)
        full = np.zeros((KD, xcols), NP_BF16)
        full[:D, : n_steps * nb] = xtc.astype(NP_BF16)
        full[D, :] = 1.0
        in_maps.append({"xt": full, "wcb": wcb})
    return in_maps


_CACHE = {}


def _run(x, Wi, Wh, bh, trace=False):
    x = np.asarray(x, np.float32)
    if "nc" not in _CACHE:
        _CACHE["nc"] = build_bass()
    nc = _CACHE["nc"]
    in_maps = host_inputs(x, Wi, Wh, bh)
    res = run_bass_kernel_spmd(nc, in_maps, list(range(NCORES)), trace=trace)
    h_full = np.concatenate(
        [np.asarray(res.results[c]["h_out"]).astype(np.float32).T for c in range(NCORES)],
        axis=0,
    )  # [B, H]
    return h_full, res


def kernel(x, Wi, Wh, bh, Wo, bo):
    x = np.asarray(x, np.float32)
    Wi = np.asarray(Wi, np.float32)
    Wh = np.asarray(Wh, np.float32)
    bh = np.asarray(bh, np.float32)
    Wo = np.asarray(Wo, np.float32)
    bo = np.asarray(bo, np.float32)
    h_full, _ = _run(x, Wi, Wh, bh)
    return (h_full @ Wo + bo).astype(np.float32)


# revision 41
# speedup vs baseline: 9.5437x; 1.1595x over previous
"""LSTM kernel for Trainium2 (Bass/Tile), 8-core data-parallel.

Model (per reference):
    xg = einsum('bsd,dg->sbg', x, Wi)            # input projections
    per step: z = xg_t + h @ Wh + bh
              i,f,g,o = split(z); c = sig(f)*c + sig(i)*tanh(g); h = sig(o)*tanh(c)
    out = h_last @ Wo + bo
Sharding: batch 256 -> 32 per core, weights replicated.

On-chip layout (per core):
  - gates-on-partitions: partition p = hidden feature; i,f,o,g,c,h are all
    [H=128, B=32] tiles and h is directly the next matmul's rhs.
  - PSUM: per 16-step chunk slot (2 slots ping-pong), PSUM bank gb holds gate
    block gb for the whole chunk: cols gb*512 + t*32 + b. One xg matmul per
    gate block covers the entire bank (N=512, start=True zeroes it); the
    per-step recurrence matmuls accumulate 32-col slices on top (start=False).
  - all matmul operands are bf16 (1 cycle/row on PE vs 4 for fp32); PSUM fp32.
  - the g-gate weights are pre-scaled by 2 on the host so tanh(zg) =
    2*sigmoid(2*zg) - 1 comes out of the SAME sigmoid activation as i,f,o
    (one ACT instr over all 4 gate blocks); the affine fixup folds into the
    DVE ops:  u = (g' - 0.5)*i ;  c = 2*u + f*c  (scalar_tensor_tensor).
  - xg matmuls for chunk n+1 are slipped one-per-4-steps into chunk n's
    recurrence so the in-order PE queue never stalls on an xg burst.
"""

import copy

import numpy as np

import concourse.bass as bass
import concourse.mybir as mybir
from concourse import tile
from concourse.bass_utils import run_bass_kernel_spmd

F32 = mybir.dt.float32
BF16 = mybir.dt.bfloat16
NP_BF16 = mybir.dt.np(mybir.dt.bfloat16)

B, S, D, H = 256, 4096, 64, 128
G4 = 4 * H  # 512
NCORES = 8
BC = B // NCORES  # 32 batch per core
TC = 16  # timesteps per PSUM chunk (4 banks)
BODY_CH = 16  # chunks per loop body (static x-slot / psum ping-pong)
KD = D + 1  # contraction rows for input projection (ones row folds bh in)
CPC = TC * BC  # x columns per chunk (512)

# on-chip gate block order [i, f, o, g]; reference order is [i, f, g, o]
_PERM = np.concatenate(
    [np.arange(0, 128), np.arange(128, 256), np.arange(384, 512), np.arange(256, 384)]
)


def _strip_self_waits(nc):
    """Drop semaphore waits an instruction holds on its OWN engine's sem.

    Engines dispatch from an in-order queue (head-of-line), so program order
    already sequences same-engine instructions; these waits only add hoisted
    EventSemaphore sequencer instructions (and wait for the producer's
    write-ack pipeline to drain). DMA and sequencer ops keep their waits.
    """
    f = nc.m.functions[0]
    for blk in f.blocks:
        for inst in blk.instructions:
            nm = type(inst).__name__
            if nm in ("InstDMACopy", "InstEventSemaphore", "InstISA", "InstTriggerDma"):
                continue
            si = inst.sync_info
            if si is None or not si.on_wait:
                continue
            eng = str(inst.engine).replace("EngineType.", "")
            keep = [
                w
                for w in si.on_wait
                if not (
                    getattr(w, "sync_type", None) == "semaphore"
                    and isinstance(getattr(w, "ant_name", None), str)
                    and w.ant_name.startswith(eng + "_")
                )
            ]
            if len(keep) != len(si.on_wait):
                inst.sync_info = mybir.SyncInfo(
                    on_wait=keep, on_update=list(si.on_update)
                )


def _transitive_strip(nc):
    """Remove semaphore waits already implied by other waits (per block).

    Soundness: sems only increase. If X waits on (S >= v), then at X's
    dispatch the instruction whose update made S reach v had completed, so
    every wait THAT instruction held (and, recursively, its same-engine
    predecessors' waits, which were satisfied at their earlier dispatches)
    is also satisfied. Tile emits each dependency directly and skips this
    reduction; each removed wait saves a hoisted EventSemaphore sequencer
    instruction on hardware (walrus allows one wait per TPB instruction).
    Facts are tracked per block only; waits on counts not produced in the
    same block are kept.
    """
    import re

    # only engine/DMA tile sems are increment-only; barrier sems decrement,
    # breaking the monotonicity the implication argument rests on
    mono = re.compile(r"^(PE|Activation|DVE|Pool|SP|DMAHW\d*)_\d+$")

    f = nc.m.functions[0]
    for blk in f.blocks:
        counts = {}  # sem name -> updates seen so far in this block
        after = {}  # sem name -> list of (count_after, guarantees dict)
        g_eng = {}  # engine -> guarantees inherited by next instr on it

        def facts(w):
            """All sem facts implied by wait w being satisfied."""
            s, v = getattr(w, "ant_name", None), w.wait_value
            if (
                getattr(w, "sync_type", None) != "semaphore"
                or not isinstance(s, str)
                or not mono.match(s)
            ):
                return {}
            out = {s: v}
            for cnt, ga in after.get(s, ()):
                if cnt >= v:
                    for k2, v2 in ga.items():
                        if out.get(k2, -1) < v2:
                            out[k2] = v2
                    break
            return out

        def implied(w, g):
            s, v = getattr(w, "ant_name", None), w.wait_value
            return (
                getattr(w, "sync_type", None) == "semaphore"
                and isinstance(s, str)
                and mono.match(s)
                and g.get(s, -1) >= v
            )

        for inst in blk.instructions:
            si = inst.sync_info
            eng = inst.engine
            g = dict(g_eng.get(eng, ()))
            if si is not None and si.on_wait:
                waits = list(si.on_wait)
                # drop waits implied by inherited facts + the OTHER kept waits
                changed = True
                while changed:
                    changed = False
                    for i, w in enumerate(waits):
                        g_i = dict(g)
                        for j, wj in enumerate(waits):
                            if j == i:
                                continue
                            for k2, v2 in facts(wj).items():
                                if g_i.get(k2, -1) < v2:
                                    g_i[k2] = v2
                        if implied(w, g_i):
                            waits.pop(i)
                            changed = True
                            break
                if len(waits) != len(si.on_wait):
                    inst.sync_info = mybir.SyncInfo(
                        on_wait=waits, on_update=list(si.on_update)
                    )
                for w in waits:
                    for k2, v2 in facts(w).items():
                        if g.get(k2, -1) < v2:
                            g[k2] = v2
            if si is not None:
                for u in si.on_update:
                    s = getattr(u, "ant_name", None)
                    if not isinstance(s, str) or not mono.match(s):
                        continue
                    counts[s] = counts.get(s, 0) + (u.update_value or 1)
                    after.setdefault(s, []).append((counts[s], dict(g)))
            g_eng[eng] = g


def _legalize_for_walrus(nc):
    """Make the Tile-scheduled module lowerable by this walrus build.

    (1) This walrus accepts only ONE semaphore wait per TPB instruction
        (e.g. Matmult/LDWEIGHTS and DMACopy structs have a single wait slot);
        Tile emits multi-wait instructions. Hoist excess waits onto standalone
        EventSemaphore sequencer instructions placed just before, on the same
        engine — semantically identical (the sequencer blocks in order).
    (2) Drop the trailing EVENT_SEMAPHORE_RANGE_CLEAR InstISA (sem-recycling
        hygiene) which this walrus cannot lower at all.
    """
    f = nc.m.functions[0]
    template = None
    for blk in f.blocks:
        for inst in blk.instructions:
            if type(inst).__name__ == "InstEventSemaphore":
                template = inst
                break
        if template is not None:
            break
    assert template is not None, "no EventSemaphore to clone"
    uid = 0
    for blk in f.blocks:
        out = []
        for inst in blk.instructions:
            nm = type(inst).__name__
            if nm == "InstISA":
                continue  # (2)
            si = inst.sync_info
            waits = list(si.on_wait) if si is not None else []
            if nm != "InstEventSemaphore" and len(waits) > 1:
                for w in waits[1:]:
                    es = copy.deepcopy(template)
                    es.name = f"{inst.name}_hoist{uid}"
                    uid += 1
                    es.engine = inst.engine
                    es.sync_info = mybir.SyncInfo(on_wait=[w], on_update=[])
                    out.append(es)
                inst.sync_info = mybir.SyncInfo(
                    on_wait=waits[:1], on_update=list(si.on_update)
                )
            out.append(inst)
        blk.instructions = out


def build_bass(
    n_steps=S,
    legalize=True,
    strip_self=True,
    transitive=True,
    xg_split=1,
    body_ch=BODY_CH,
):
    n_ch = n_steps // TC
    assert n_ch % body_ch == 0 and n_steps % TC == 0 and body_ch % 2 == 0
    n_iter = n_ch // body_ch
    pad_ch = n_ch + body_ch + 1
    xcols = pad_ch * CPC

    nc = bass.Bass()
    xt = nc.declare_dram_parameter("xt", [KD, xcols], BF16, isOutput=False)
    # combined weights: cols [0:512] = Wh (permuted), cols [512:1024] = [Wi; bh]
    # (rows 65:128 of the right half are zero padding); g blocks pre-scaled x2
    wcb = nc.declare_dram_parameter("wcb", [H, 2 * G4], BF16, isOutput=False)
    hout = nc.declare_dram_parameter("h_out", [H, BC], F32, isOutput=True)

    with tile.TileContext(nc) as tc:
        with (
            tc.tile_pool(name="weights", bufs=1) as wpool,
            tc.tile_pool(name="xin", bufs=1) as xpool,
            tc.tile_pool(name="state", bufs=1) as spool,
            tc.tile_pool(name="psum", bufs=1, space=bass.MemorySpace.PSUM) as ppool,
        ):
            w_sb = wpool.tile([H, 2 * G4], BF16, tag="w")
            wh_sb = w_sb[:, 0:G4]
            wi_sb = w_sb[:KD, G4 : 2 * G4]
            xs_all = xpool.tile([KD, body_ch * CPC], BF16, tag="xs")
            xs = [xs_all[:, k * CPC : (k + 1) * CPC] for k in range(body_ch)]
            # persistent state: st = sigmoid outputs [i|f|o|g'] (bf16),
            # cc = cell state (fp32), wk = [u|v] (fp32), tc_sb = tanh(c) (bf16)
            st = spool.tile([H, 4 * BC], BF16, tag="st")
            cc = spool.tile([H, BC], F32, tag="cc")
            wk = spool.tile([H, 2 * BC], F32, tag="wk")
            tc_sb = spool.tile([H, BC], BF16, tag="tc")
            h_sb = spool.tile([H, BC], BF16, tag="h")
            hf_sb = spool.tile([H, BC], F32, tag="hf")
            ps = [
                ppool.tile([H, TC * 128], F32, tag=f"ps{k}", name=f"ps{k}")
                for k in range(2)
            ]

            def xg_items(pi, xsrc):
                """The input-projection matmuls for one chunk into psum slot
                pi (each bank = one gate block; first write start=True zeroes
                it), as thunks. xg_split subdivides each bank's matmul."""
                items = []
                w = 512 // xg_split
                for gb in range(4):
                    for s in range(xg_split):

                        def mm(gb=gb, s=s):
                            nc.tensor.matmul(
                                ps[pi][
                                    :, gb * 512 + s * w : gb * 512 + (s + 1) * w
                                ],
                                wi_sb[:, gb * H : (gb + 1) * H],
                                xsrc[:, s * w : (s + 1) * w],
                                start=(s == 0),
                                stop=False,
                                skip_group_check=True,
                            )

                        items.append(mm)
                return items

            def xg_chunk(pi, xsrc):
                for it in xg_items(pi, xsrc):
                    it()

            def step(pi, j):
                """One LSTM timestep; z for step j lives at cols j*32 of each
                gate bank."""
                p = ps[pi]
                act = mybir.ActivationFunctionType
                alu = mybir.AluOpType
                for gb in range(4):
                    nc.tensor.matmul(
                        p[:, gb * 512 + j * BC : gb * 512 + (j + 1) * BC],
                        wh_sb[:, gb * H : (gb + 1) * H],
                        h_sb[:, :],
                        start=False,
                        stop=True,
                        skip_group_check=True,
                    )
                # strided view: the four gate blocks for step j sit 512 apart
                pz = p[:].rearrange("p (gb t z) -> p gb t z", gb=4, t=TC)
                # one sigmoid over [i|f|o|g'] (g weights pre-scaled x2 on host)
                nc.scalar.activation(
                    st[:].rearrange("p (a z) -> p a z", z=BC)[:, :, :],
                    pz[:, :, j, :],
                    act.Sigmoid,
                )
                # v = f * c  (reads previous c)
                nc.vector.tensor_mul(wk[:, BC : 2 * BC], st[:, BC : 2 * BC], cc[:, :])
                # u = (g' - 0.5) * i   [= 0.5 * i * tanh(zg)]
                nc.vector.scalar_tensor_tensor(
                    wk[:, 0:BC], st[:, 3 * BC : 4 * BC], 0.5, st[:, 0:BC],
                    alu.subtract, alu.mult,
                )
                # c = 2*u + v
                nc.vector.scalar_tensor_tensor(
                    cc[:, :], wk[:, 0:BC], 2.0, wk[:, BC : 2 * BC],
                    alu.mult, alu.add,
                )
                nc.scalar.activation(tc_sb[:, :], cc[:, :], act.Tanh)
                # h = o * tanh(c)
                nc.vector.tensor_mul(h_sb[:, :], st[:, 2 * BC : 3 * BC], tc_sb[:, :])

            def rec_chunk(pi, xg=()):
                """16 recurrence steps on slot pi; one xg matmul for the OTHER
                psum slot's next chunk is slipped in per 4 steps so the PE
                queue never sees a blocking burst at a chunk edge (tile-level
                WAR on the previous chunk's sigmoid reads keeps the bank
                zeroing correctly ordered)."""
                done = 0
                for j in range(TC):
                    step(pi, j)
                    want = (j + 1) * len(xg) // TC
                    while done < want:
                        xg[done]()
                        done += 1

            # ---- preamble ----
            nc.sync.dma_start(w_sb[:], wcb[:])
            nc.vector.memset(h_sb[:], 0.0)
            nc.vector.memset(cc[:], 0.0)
            nc.sync.dma_start(xs_all[:], xt[:, 0 : body_ch * CPC])
            xg_chunk(0, xs[0])

            # ---- main loop: body covers chunks B*i .. B*i+B-1; rec of chunk
            # n carries the xg matmuls of chunk n+1 (on the other psum slot);
            # the x slot consumed by next body's chunk m is refilled right
            # after its last reader (the xg carried by rec line m-1), a full
            # body ahead of its consumer ----
            with tc.For_i(
                0, n_iter, 1, hint_engines=(mybir.EngineType.PE,)
            ) as iv:
                base = iv * (body_ch * CPC)

                assert body_ch >= 8 and body_ch % 4 == 0
                for m in range(body_ch):
                    rec_chunk(
                        m % 2, xg_items((m + 1) % 2, xs[(m + 1) % body_ch])
                    )
                    # after line 4k+3, slots [4k..4k+3] have had their last
                    # read (lines 4k-1..4k+2); refill them with the chunks
                    # they'll serve next body (>= 4-line lead to consumers)
                    if m % 4 == 3:
                        g0 = m - 3
                        nc.sync.dma_start(
                            xs_all[:, g0 * CPC : (g0 + 4) * CPC],
                            xt[:, bass.ds(base + (body_ch + g0) * CPC, 4 * CPC)],
                        )

            # widen the final h to fp32 for the output DMA
            nc.vector.tensor_scalar_add(hf_sb[:, :], h_sb[:, :], 0.0)
            nc.sync.dma_start(hout[:], hf_sb[:])

    if strip_self:
        _strip_self_waits(nc)
    if transitive:
        _transitive_strip(nc)
    if legalize:  # CoreSim can't run the post-hoc clones; HW compile needs them
        _legalize_for_walrus(nc)
    return nc


def host_inputs(x, Wi, Wh, bh, n_steps=S, body_ch=BODY_CH):
    """Per-core input maps: transposed/padded x (bf16), permuted weights (bf16,
    g block pre-scaled by 2 for the tanh-via-sigmoid trick)."""
    n_ch = n_steps // TC
    pad_ch = n_ch + body_ch + 1
    xcols = pad_ch * CPC
    gscale = np.ones((G4,), np.float32)
    gscale[384:512] = 2.0  # post-perm cols 384:512 are the g block
    wcb = np.zeros((H, 2 * G4), np.float32)
    wcb[:, 0:G4] = Wh[:, _PERM] * gscale
    wcb[0:D, G4:] = Wi[:, _PERM] * gscale
    wcb[D, G4:] = bh[_PERM] * gscale
    wcb = wcb.astype(NP_BF16)
    nb = x.shape[0] // NCORES
    in_maps = []
    for core in range(NCORES):
        xc = x[core * nb : (core + 1) * nb]  # [BC, n_steps, D]
        xtc = np.ascontiguousarray(xc.transpose(2, 1, 0)).reshape(D, n_steps * nb)
        full = np.zeros((KD, xcols), NP_BF16)
        full[:D, : n_steps * nb] = xtc.astype(NP_BF16)
        full[D, :] = 1.0
        in_maps.append({"xt": full, "wcb": wcb})
    return in_maps


_CACHE = {}


def _run(x, Wi, Wh, bh, trace=False):
    x = np.asarray(x, np.float32)
    if "nc" not in _CACHE:
        _CACHE["nc"] = build_bass()
    nc = _CACHE["nc"]
    in_maps = host_inputs(x, Wi, Wh, bh)
    res = run_bass_kernel_spmd(nc, in_maps, list(range(NCORES)), trace=trace)
    h_full = np.concatenate(
        [np.asarray(res.results[c]["h_out"]).astype(np.float32).T for c in range(NCORES)],
        axis=0,
    )  # [B, H]
    return h_full, res


def kernel(x, Wi, Wh, bh, Wo, bo):
    x = np.asarray(x, np.float32)
    Wi = np.asarray(Wi, np.float32)
    Wh = np.asarray(Wh, np.float32)
    bh = np.asarray(bh, np.float32)
    Wo = np.asarray(Wo, np.float32)
    bo = np.asarray(bo, np.float32)
    h_full, _ = _run(x, Wi, Wh, bh)
    return (h_full @ Wo + bo).astype(np.float32)


# revision 44
# speedup vs baseline: 10.0216x; 1.0501x over previous
"""LSTM kernel for Trainium2 (Bass/Tile), 8-core data-parallel.

Model (per reference):
    xg = einsum('bsd,dg->sbg', x, Wi)            # input projections
    per step: z = xg_t + h @ Wh + bh
              i,f,g,o = split(z); c = sig(f)*c + sig(i)*tanh(g); h = sig(o)*tanh(c)
    out = h_last @ Wo + bo
Sharding: batch 256 -> 32 per core, weights replicated.

On-chip layout (per core):
  - gates-on-partitions: partition p = hidden feature; i,f,o,g,c,h are all
    [H=128, B=32] tiles and h is directly the next matmul's rhs.
  - PSUM: per 16-step chunk slot (2 slots ping-pong), PSUM bank gb holds gate
    block gb for the whole chunk: cols gb*512 + t*32 + b. One xg matmul per
    gate block covers the entire bank (N=512, start=True zeroes it); the
    per-step recurrence matmuls accumulate 32-col slices on top (start=False).
  - all matmul operands are bf16 (1 cycle/row on PE vs 4 for fp32); PSUM fp32.
  - the g-gate weights are pre-scaled by 2 on the host so tanh(zg) =
    2*sigmoid(2*zg) - 1 comes out of the SAME sigmoid activation as i,f,o
    (one ACT instr over all 4 gate blocks); the affine fixup folds into the
    DVE ops:  u = (g' - 0.5)*i ;  c = 2*u + f*c  (scalar_tensor_tensor).
  - xg matmuls for chunk n+1 are slipped one-per-4-steps into chunk n's
    recurrence so the in-order PE queue never stalls on an xg burst.
"""

import copy

import numpy as np

import concourse.bass as bass
import concourse.mybir as mybir
from concourse import tile
from concourse.bass_utils import run_bass_kernel_spmd

F32 = mybir.dt.float32
BF16 = mybir.dt.bfloat16
NP_BF16 = mybir.dt.np(mybir.dt.bfloat16)

B, S, D, H = 256, 4096, 64, 128
G4 = 4 * H  # 512
NCORES = 8
BC = B // NCORES  # 32 batch per core
TC = 16  # timesteps per PSUM chunk (4 banks)
BODY_CH = 16  # chunks per loop body (static x-slot / psum ping-pong)
KD = D + 1  # contraction rows for input projection (ones row folds bh in)
CPC = TC * BC  # x columns per chunk (512)

# on-chip gate block order [i, f, o, g]; reference order is [i, f, g, o]
_PERM = np.concatenate(
    [np.arange(0, 128), np.arange(128, 256), np.arange(384, 512), np.arange(256, 384)]
)


# DVE reorders/overlaps ready instructions (proven empirically: stripping
# its self-waits corrupts results); every other engine dispatches strictly
# in order, so their self-waits are redundant.
_STRIP_ENGINES = ("Activation", "PE", "Pool", "SP")


def _strip_self_waits(nc):
    """Drop semaphore waits an instruction holds on its OWN engine's sem.

    Engines dispatch from an in-order queue (head-of-line), so program order
    already sequences same-engine instructions; these waits only add hoisted
    EventSemaphore sequencer instructions (and wait for the producer's
    write-ack pipeline to drain). DMA and sequencer ops keep their waits.
    """
    f = nc.m.functions[0]
    for blk in f.blocks:
        for inst in blk.instructions:
            nm = type(inst).__name__
            if nm in ("InstDMACopy", "InstEventSemaphore", "InstISA", "InstTriggerDma"):
                continue
            si = inst.sync_info
            if si is None or not si.on_wait:
                continue
            eng = str(inst.engine).replace("EngineType.", "")
            if eng not in _STRIP_ENGINES:
                continue
            keep = [
                w
                for w in si.on_wait
                if not (
                    getattr(w, "sync_type", None) == "semaphore"
                    and isinstance(getattr(w, "ant_name", None), str)
                    and w.ant_name.startswith(eng + "_")
                )
            ]
            if len(keep) != len(si.on_wait):
                inst.sync_info = mybir.SyncInfo(
                    on_wait=keep, on_update=list(si.on_update)
                )


def _transitive_strip(nc):
    """Remove semaphore waits already implied by other waits (per block).

    Soundness: sems only increase. If X waits on (S >= v), then at X's
    dispatch the instruction whose update made S reach v had completed, so
    every wait THAT instruction held (and, recursively, its same-engine
    predecessors' waits, which were satisfied at their earlier dispatches)
    is also satisfied. Tile emits each dependency directly and skips this
    reduction; each removed wait saves a hoisted EventSemaphore sequencer
    instruction on hardware (walrus allows one wait per TPB instruction).
    Facts are tracked per block only; waits on counts not produced in the
    same block are kept.
    """
    import re

    # only engine/DMA tile sems are increment-only; barrier sems decrement,
    # breaking the monotonicity the implication argument rests on
    mono = re.compile(r"^(PE|Activation|DVE|Pool|SP|DMAHW\d*)_\d+$")

    f = nc.m.functions[0]
    for blk in f.blocks:
        counts = {}  # sem name -> updates seen so far in this block
        after = {}  # sem name -> list of (count_after, guarantees dict)
        g_eng = {}  # engine -> guarantees inherited by next instr on it

        def facts(w):
            """All sem facts implied by wait w being satisfied."""
            s, v = getattr(w, "ant_name", None), w.wait_value
            if (
                getattr(w, "sync_type", None) != "semaphore"
                or not isinstance(s, str)
                or not mono.match(s)
            ):
                return {}
            out = {s: v}
            for cnt, ga in after.get(s, ()):
                if cnt >= v:
                    for k2, v2 in ga.items():
                        if out.get(k2, -1) < v2:
                            out[k2] = v2
                    break
            return out

        def implied(w, g):
            s, v = getattr(w, "ant_name", None), w.wait_value
            return (
                getattr(w, "sync_type", None) == "semaphore"
                and isinstance(s, str)
                and mono.match(s)
                and g.get(s, -1) >= v
            )

        for inst in blk.instructions:
            si = inst.sync_info
            eng = inst.engine
            # DVE may execute a ready instruction before an earlier waiting
            # one, so facts do NOT flow along its program order
            g = {} if str(eng).endswith("DVE") else dict(g_eng.get(eng, ()))
            if si is not None and si.on_wait:
                waits = list(si.on_wait)
                # drop waits implied by inherited facts + the OTHER kept waits
                changed = True
                while changed:
                    changed = False
                    for i, w in enumerate(waits):
                        g_i = dict(g)
                        for j, wj in enumerate(waits):
                            if j == i:
                                continue
                            for k2, v2 in facts(wj).items():
                                if g_i.get(k2, -1) < v2:
                                    g_i[k2] = v2
                        if implied(w, g_i):
                            waits.pop(i)
                            changed = True
                            break
                if len(waits) != len(si.on_wait):
                    inst.sync_info = mybir.SyncInfo(
                        on_wait=waits, on_update=list(si.on_update)
                    )
                for w in waits:
                    for k2, v2 in facts(w).items():
                        if g.get(k2, -1) < v2:
                            g[k2] = v2
            if si is not None:
                for u in si.on_update:
                    s = getattr(u, "ant_name", None)
                    if not isinstance(s, str) or not mono.match(s):
                        continue
                    counts[s] = counts.get(s, 0) + (u.update_value or 1)
                    after.setdefault(s, []).append((counts[s], dict(g)))
            g_eng[eng] = g


def _legalize_for_walrus(nc):
    """Make the Tile-scheduled module lowerable by this walrus build.

    (1) This walrus accepts only ONE semaphore wait per TPB instruction
        (e.g. Matmult/LDWEIGHTS and DMACopy structs have a single wait slot);
        Tile emits multi-wait instructions. Hoist excess waits onto standalone
        EventSemaphore sequencer instructions placed just before, on the same
        engine — semantically identical (the sequencer blocks in order).
    (2) Drop the trailing EVENT_SEMAPHORE_RANGE_CLEAR InstISA (sem-recycling
        hygiene) which this walrus cannot lower at all.
    """
    f = nc.m.functions[0]
    template = None
    for blk in f.blocks:
        for inst in blk.instructions:
            if type(inst).__name__ == "InstEventSemaphore":
                template = inst
                break
        if template is not None:
            break
    assert template is not None, "no EventSemaphore to clone"
    uid = 0
    for blk in f.blocks:
        out = []
        for inst in blk.instructions:
            nm = type(inst).__name__
            if nm == "InstISA":
                continue  # (2)
            si = inst.sync_info
            waits = list(si.on_wait) if si is not None else []
            if nm != "InstEventSemaphore" and len(waits) > 1:
                for w in waits[1:]:
                    es = copy.deepcopy(template)
                    es.name = f"{inst.name}_hoist{uid}"
                    uid += 1
                    es.engine = inst.engine
                    es.sync_info = mybir.SyncInfo(on_wait=[w], on_update=[])
                    out.append(es)
                inst.sync_info = mybir.SyncInfo(
                    on_wait=waits[:1], on_update=list(si.on_update)
                )
            out.append(inst)
        blk.instructions = out


def build_bass(
    n_steps=S,
    legalize=True,
    strip_self=True,
    transitive=True,
    xg_split=1,
    body_ch=BODY_CH,
):
    n_ch = n_steps // TC
    body_ch = min(body_ch, n_ch)
    assert n_ch % body_ch == 0 and n_steps % TC == 0 and body_ch % 2 == 0
    n_iter = n_ch // body_ch
    pad_ch = n_ch + body_ch + 1
    xcols = pad_ch * CPC

    nc = bass.Bass()
    xt = nc.declare_dram_parameter("xt", [KD, xcols], BF16, isOutput=False)
    # combined weights: cols [0:512] = Wh (permuted), cols [512:1024] = [Wi; bh]
    # (rows 65:128 of the right half are zero padding); g blocks pre-scaled x2
    wcb = nc.declare_dram_parameter("wcb", [H, 2 * G4], BF16, isOutput=False)
    hout = nc.declare_dram_parameter("h_out", [H, BC], F32, isOutput=True)

    with tile.TileContext(nc) as tc:
        with (
            tc.tile_pool(name="weights", bufs=1) as wpool,
            tc.tile_pool(name="xin", bufs=1) as xpool,
            tc.tile_pool(name="state", bufs=1) as spool,
            tc.tile_pool(name="psum", bufs=1, space=bass.MemorySpace.PSUM) as ppool,
        ):
            w_sb = wpool.tile([H, 2 * G4], BF16, tag="w")
            wh_sb = w_sb[:, 0:G4]
            wi_sb = w_sb[:KD, G4 : 2 * G4]
            xs_all = xpool.tile([KD, body_ch * CPC], BF16, tag="xs")
            xs = [xs_all[:, k * CPC : (k + 1) * CPC] for k in range(body_ch)]
            # persistent state: st = sigmoid outputs [i|f|o|g'] (bf16),
            # cc = cell state (fp32), wk = [u|v] (fp32), tc_sb = tanh(c) (bf16)
            st = spool.tile([H, 4 * BC], BF16, tag="st")
            cc = spool.tile([H, BC], F32, tag="cc")
            wk = spool.tile([H, 2 * BC], F32, tag="wk")
            tc_sb = spool.tile([H, BC], BF16, tag="tc")
            h_sb = spool.tile([H, BC], BF16, tag="h")
            hf_sb = spool.tile([H, BC], F32, tag="hf")
            ps = [
                ppool.tile([H, TC * 128], F32, tag=f"ps{k}", name=f"ps{k}")
                for k in range(2)
            ]

            def xg_items(pi, xsrc):
                """The input-projection matmuls for one chunk into psum slot
                pi (each bank = one gate block; first write start=True zeroes
                it), as thunks. xg_split subdivides each bank's matmul."""
                items = []
                w = 512 // xg_split
                for gb in range(4):
                    for s in range(xg_split):

                        def mm(gb=gb, s=s):
                            nc.tensor.matmul(
                                ps[pi][
                                    :, gb * 512 + s * w : gb * 512 + (s + 1) * w
                                ],
                                wi_sb[:, gb * H : (gb + 1) * H],
                                xsrc[:, s * w : (s + 1) * w],
                                start=(s == 0),
                                stop=False,
                                skip_group_check=True,
                            )

                        items.append(mm)
                return items

            def xg_chunk(pi, xsrc):
                for it in xg_items(pi, xsrc):
                    it()

            def step(pi, j):
                """One LSTM timestep; z for step j lives at cols j*32 of each
                gate bank."""
                p = ps[pi]
                act = mybir.ActivationFunctionType
                alu = mybir.AluOpType
                for gb in range(4):
                    nc.tensor.matmul(
                        p[:, gb * 512 + j * BC : gb * 512 + (j + 1) * BC],
                        wh_sb[:, gb * H : (gb + 1) * H],
                        h_sb[:, :],
                        start=False,
                        stop=True,
                        skip_group_check=True,
                    )
                # strided view: the four gate blocks for step j sit 512 apart
                pz = p[:].rearrange("p (gb t z) -> p gb t z", gb=4, t=TC)
                # one sigmoid over [i|f|o|g'] (g weights pre-scaled x2 on host)
                nc.scalar.activation(
                    st[:].rearrange("p (a z) -> p a z", z=BC)[:, :, :],
                    pz[:, :, j, :],
                    act.Sigmoid,
                )
                # v = f * c  (reads previous c)
                nc.vector.tensor_mul(wk[:, BC : 2 * BC], st[:, BC : 2 * BC], cc[:, :])
                # u = (g' - 0.5) * i   [= 0.5 * i * tanh(zg)]
                nc.vector.scalar_tensor_tensor(
                    wk[:, 0:BC], st[:, 3 * BC : 4 * BC], 0.5, st[:, 0:BC],
                    alu.subtract, alu.mult,
                )
                # c = 2*u + v
                nc.vector.scalar_tensor_tensor(
                    cc[:, :], wk[:, 0:BC], 2.0, wk[:, BC : 2 * BC],
                    alu.mult, alu.add,
                )
                nc.scalar.activation(tc_sb[:, :], cc[:, :], act.Tanh)
                # h = o * tanh(c)
                nc.vector.tensor_mul(h_sb[:, :], st[:, 2 * BC : 3 * BC], tc_sb[:, :])

            def rec_chunk(pi, xg=()):
                """16 recurrence steps on slot pi; one xg matmul for the OTHER
                psum slot's next chunk is slipped in per 4 steps so the PE
                queue never sees a blocking burst at a chunk edge (tile-level
                WAR on the previous chunk's sigmoid reads keeps the bank
                zeroing correctly ordered)."""
                done = 0
                for j in range(TC):
                    step(pi, j)
                    want = (j + 1) * len(xg) // TC
                    while done < want:
                        xg[done]()
                        done += 1

            # ---- preamble ----
            nc.sync.dma_start(w_sb[:], wcb[:])
            nc.vector.memset(h_sb[:], 0.0)
            nc.vector.memset(cc[:], 0.0)
            nc.sync.dma_start(xs_all[:], xt[:, 0 : body_ch * CPC])
            xg_chunk(0, xs[0])

            # ---- main loop: body covers chunks B*i .. B*i+B-1; rec of chunk
            # n carries the xg matmuls of chunk n+1 (on the other psum slot);
            # the x slot consumed by next body's chunk m is refilled right
            # after its last reader (the xg carried by rec line m-1), a full
            # body ahead of its consumer ----
            with tc.For_i(
                0, n_iter, 1, hint_engines=(mybir.EngineType.PE,)
            ) as iv:
                base = iv * (body_ch * CPC)

                assert body_ch >= 8 and body_ch % 4 == 0
                for m in range(body_ch):
                    rec_chunk(
                        m % 2, xg_items((m + 1) % 2, xs[(m + 1) % body_ch])
                    )
                    # after line 4k+3, slots [4k..4k+3] have had their last
                    # read (lines 4k-1..4k+2); refill them with the chunks
                    # they'll serve next body (>= 4-line lead to consumers)
                    if m % 4 == 3:
                        g0 = m - 3
                        nc.sync.dma_start(
                            xs_all[:, g0 * CPC : (g0 + 4) * CPC],
                            xt[:, bass.ds(base + (body_ch + g0) * CPC, 4 * CPC)],
                        )

            # widen the final h to fp32 for the output DMA
            nc.vector.tensor_scalar_add(hf_sb[:, :], h_sb[:, :], 0.0)
            nc.sync.dma_start(hout[:], hf_sb[:])

    if strip_self:
        _strip_self_waits(nc)
    if transitive:
        _transitive_strip(nc)
    if legalize:  # CoreSim can't run the post-hoc clones; HW compile needs them
        _legalize_for_walrus(nc)
    return nc


def host_inputs(x, Wi, Wh, bh, n_steps=S, body_ch=BODY_CH):
    """Per-core input maps: transposed/padded x (bf16), permuted weights (bf16,
    g block pre-scaled by 2 for the tanh-via-sigmoid trick)."""
    n_ch = n_steps // TC
    body_ch = min(body_ch, n_ch)
    pad_ch = n_ch + body_ch + 1
    xcols = pad_ch * CPC
    gscale = np.ones((G4,), np.float32)
    gscale[384:512] = 2.0  # post-perm cols 384:512 are the g block
    wcb = np.zeros((H, 2 * G4), np.float32)
    wcb[:, 0:G4] = Wh[:, _PERM] * gscale
    wcb[0:D, G4:] = Wi[:, _PERM] * gscale
    wcb[D, G4:] = bh[_PERM] * gscale
    wcb = wcb.astype(NP_BF16)
    nb = x.shape[0] // NCORES
    in_maps = []
    for core in range(NCORES):
        xc = x[core * nb : (core + 1) * nb]  # [BC, n_steps, D]
        xtc = np.ascontiguousarray(xc.transpose(2, 1, 0)).reshape(D, n_steps * nb)
        full = np.zeros((KD, xcols), NP_BF16)
        full[:D, : n_steps * nb] = xtc.astype(NP_BF16)
        full[D, :] = 1.0
        in_maps.append({"xt": full, "wcb": wcb})
    return in_maps


_CACHE = {}


def _run(x, Wi, Wh, bh, trace=False):
    x = np.asarray(x, np.float32)
    if "nc" not in _CACHE:
        _CACHE["nc"] = build_bass()
    nc = _CACHE["nc"]
    in_maps = host_inputs(x, Wi, Wh, bh)
    res = run_bass_kernel_spmd(nc, in_maps, list(range(NCORES)), trace=trace)
    h_full = np.concatenate(
        [np.asarray(res.results[c]["h_out"]).astype(np.float32).T for c in range(NCORES)],
        axis=0,
    )  # [B, H]
    return h_full, res


def kernel(x, Wi, Wh, bh, Wo, bo):
    x = np.asarray(x, np.float32)
    Wi = np.asarray(Wi, np.float32)
    Wh = np.asarray(Wh, np.float32)
    bh = np.asarray(bh, np.float32)
    Wo = np.asarray(Wo, np.float32)
    bo = np.asarray(bo, np.float32)
    h_full, _ = _run(x, Wi, Wh, bh)
    return (h_full @ Wo + bo).astype(np.float32)


# revision 49
# speedup vs baseline: 10.1379x; 1.0116x over previous
"""LSTM kernel for Trainium2 (Bass/Tile), 8-core data-parallel.

Model (per reference):
    xg = einsum('bsd,dg->sbg', x, Wi)            # input projections
    per step: z = xg_t + h @ Wh + bh
              i,f,g,o = split(z); c = sig(f)*c + sig(i)*tanh(g); h = sig(o)*tanh(c)
    out = h_last @ Wo + bo
Sharding: batch 256 -> 32 per core, weights replicated.

On-chip layout (per core):
  - gates-on-partitions: partition p = hidden feature; i,f,o,g,c,h are all
    [H=128, B=32] tiles and h is directly the next matmul's rhs.
  - PSUM: per 16-step chunk slot (2 slots ping-pong), PSUM bank gb holds gate
    block gb for the whole chunk: cols gb*512 + t*32 + b. One xg matmul per
    gate block covers the entire bank (N=512, start=True zeroes it); the
    per-step recurrence matmuls accumulate 32-col slices on top (start=False).
  - all matmul operands are bf16 (1 cycle/row on PE vs 4 for fp32); PSUM fp32.
  - the g-gate weights are pre-scaled by 2 on the host so tanh(zg) =
    2*sigmoid(2*zg) - 1 comes out of the SAME sigmoid activation as i,f,o
    (one ACT instr over all 4 gate blocks); the affine fixup folds into the
    DVE ops:  u = (g' - 0.5)*i ;  c = 2*u + f*c  (scalar_tensor_tensor).
  - xg matmuls for chunk n+1 are slipped one-per-4-steps into chunk n's
    recurrence so the in-order PE queue never stalls on an xg burst.
"""

import copy

import numpy as np

import concourse.bass as bass
import concourse.mybir as mybir
from concourse import tile
from concourse.bass_utils import run_bass_kernel_spmd

F32 = mybir.dt.float32
BF16 = mybir.dt.bfloat16
NP_BF16 = mybir.dt.np(mybir.dt.bfloat16)

B, S, D, H = 256, 4096, 64, 128
G4 = 4 * H  # 512
NCORES = 8
BC = B // NCORES  # 32 batch per core
TC = 16  # timesteps per PSUM chunk (4 banks)
BODY_CH = 32  # chunks per loop body (static x-slot / psum ping-pong)
KD = D + 1  # contraction rows for input projection (ones row folds bh in)
CPC = TC * BC  # x columns per chunk (512)

# on-chip gate block order [i, f, o, g]; reference order is [i, f, g, o]
_PERM = np.concatenate(
    [np.arange(0, 128), np.arange(128, 256), np.arange(384, 512), np.arange(256, 384)]
)


# DVE reorders/overlaps ready instructions (proven empirically: stripping
# its self-waits corrupts results); every other engine dispatches strictly
# in order, so their self-waits are redundant.
_STRIP_ENGINES = ("Activation", "PE", "Pool", "SP")


def _strip_self_waits(nc):
    """Drop semaphore waits an instruction holds on its OWN engine's sem.

    Engines dispatch from an in-order queue (head-of-line), so program order
    already sequences same-engine instructions; these waits only add hoisted
    EventSemaphore sequencer instructions (and wait for the producer's
    write-ack pipeline to drain). DMA and sequencer ops keep their waits.
    """
    f = nc.m.functions[0]
    for blk in f.blocks:
        for inst in blk.instructions:
            nm = type(inst).__name__
            if nm in ("InstDMACopy", "InstEventSemaphore", "InstISA", "InstTriggerDma"):
                continue
            si = inst.sync_info
            if si is None or not si.on_wait:
                continue
            eng = str(inst.engine).replace("EngineType.", "")
            if eng not in _STRIP_ENGINES:
                continue
            keep = [
                w
                for w in si.on_wait
                if not (
                    getattr(w, "sync_type", None) == "semaphore"
                    and isinstance(getattr(w, "ant_name", None), str)
                    and w.ant_name.startswith(eng + "_")
                )
            ]
            if len(keep) != len(si.on_wait):
                inst.sync_info = mybir.SyncInfo(
                    on_wait=keep, on_update=list(si.on_update)
                )


def _transitive_strip(nc):
    """Remove semaphore waits already implied by other waits (per block).

    Soundness: sems only increase. If X waits on (S >= v), then at X's
    dispatch the instruction whose update made S reach v had completed, so
    every wait THAT instruction held (and, recursively, its same-engine
    predecessors' waits, which were satisfied at their earlier dispatches)
    is also satisfied. Tile emits each dependency directly and skips this
    reduction; each removed wait saves a hoisted EventSemaphore sequencer
    instruction on hardware (walrus allows one wait per TPB instruction).
    Facts are tracked per block only; waits on counts not produced in the
    same block are kept.
    """
    import re

    # only engine/DMA tile sems are increment-only; barrier sems decrement,
    # breaking the monotonicity the implication argument rests on
    mono = re.compile(r"^(PE|Activation|DVE|Pool|SP|DMAHW\d*)_\d+$")

    f = nc.m.functions[0]
    base_counts = {}  # sem name -> total updates in prior first-pass blocks
    for blk in f.blocks:
        # wait values are absolute within an iteration: (updates before the
        # loop) + in-block index. The skip/reset blocks are not on the
        # first-iteration path; their bump NoOps must not shift the base.
        on_path = not (blk.name.endswith("_skip") or blk.name.endswith("_reset"))
        blk_base = dict(base_counts)
        counts = dict(base_counts)  # sem name -> absolute updates seen
        after = {}  # sem name -> list of (count_after, guarantees dict)
        g_eng = {}  # engine -> guarantees inherited by next instr on it

        def facts(w):
            """All sem facts implied by wait w being satisfied."""
            s, v = getattr(w, "ant_name", None), w.wait_value
            if (
                getattr(w, "sync_type", None) != "semaphore"
                or not isinstance(s, str)
                or not mono.match(s)
            ):
                return {}
            out = {s: v}
            if v <= blk_base.get(s, 0):
                return out  # producer predates this block: no richer facts
            for cnt, ga in after.get(s, ()):
                if cnt >= v:
                    for k2, v2 in ga.items():
                        if out.get(k2, -1) < v2:
                            out[k2] = v2
                    break
            return out

        def implied(w, g):
            s, v = getattr(w, "ant_name", None), w.wait_value
            return (
                getattr(w, "sync_type", None) == "semaphore"
                and isinstance(s, str)
                and mono.match(s)
                and g.get(s, -1) >= v
            )

        for inst in blk.instructions:
            si = inst.sync_info
            eng = inst.engine
            # DVE may execute a ready instruction before an earlier waiting
            # one, so facts do NOT flow along its program order
            g = {} if str(eng).endswith("DVE") else dict(g_eng.get(eng, ()))
            if si is not None and si.on_wait:
                waits = list(si.on_wait)
                # drop waits implied by inherited facts + the OTHER kept waits
                changed = True
                while changed:
                    changed = False
                    for i, w in enumerate(waits):
                        g_i = dict(g)
                        for j, wj in enumerate(waits):
                            if j == i:
                                continue
                            for k2, v2 in facts(wj).items():
                                if g_i.get(k2, -1) < v2:
                                    g_i[k2] = v2
                        if implied(w, g_i):
                            waits.pop(i)
                            changed = True
                            break
                if len(waits) != len(si.on_wait):
                    inst.sync_info = mybir.SyncInfo(
                        on_wait=waits, on_update=list(si.on_update)
                    )
                for w in waits:
                    for k2, v2 in facts(w).items():
                        if g.get(k2, -1) < v2:
                            g[k2] = v2
            if si is not None:
                for u in si.on_update:
                    s = getattr(u, "ant_name", None)
                    if not isinstance(s, str) or not mono.match(s):
                        continue
                    counts[s] = counts.get(s, 0) + (u.update_value or 1)
                    after.setdefault(s, []).append((counts[s], dict(g)))
            g_eng[eng] = g
        if on_path:
            base_counts = {
                s: c for s, c in counts.items()
            }


def _legalize_for_walrus(nc):
    """Make the Tile-scheduled module lowerable by this walrus build.

    (1) This walrus accepts only ONE semaphore wait per TPB instruction
        (e.g. Matmult/LDWEIGHTS and DMACopy structs have a single wait slot);
        Tile emits multi-wait instructions. Hoist excess waits onto standalone
        EventSemaphore sequencer instructions placed just before, on the same
        engine — semantically identical (the sequencer blocks in order).
    (2) Drop the trailing EVENT_SEMAPHORE_RANGE_CLEAR InstISA (sem-recycling
        hygiene) which this walrus cannot lower at all.
    """
    f = nc.m.functions[0]
    template = None
    for blk in f.blocks:
        for inst in blk.instructions:
            if type(inst).__name__ == "InstEventSemaphore":
                template = inst
                break
        if template is not None:
            break
    assert template is not None, "no EventSemaphore to clone"
    uid = 0
    for blk in f.blocks:
        out = []
        for inst in blk.instructions:
            nm = type(inst).__name__
            if nm == "InstISA":
                continue  # (2)
            si = inst.sync_info
            waits = list(si.on_wait) if si is not None else []
            if nm != "InstEventSemaphore" and len(waits) > 1:
                for w in waits[1:]:
                    es = copy.deepcopy(template)
                    es.name = f"{inst.name}_hoist{uid}"
                    uid += 1
                    es.engine = inst.engine
                    es.sync_info = mybir.SyncInfo(on_wait=[w], on_update=[])
                    out.append(es)
                inst.sync_info = mybir.SyncInfo(
                    on_wait=waits[:1], on_update=list(si.on_update)
                )
            out.append(inst)
        blk.instructions = out


def build_bass(
    n_steps=S,
    legalize=True,
    strip_self=True,
    transitive=True,
    xg_split=1,
    body_ch=BODY_CH,
):
    n_ch = n_steps // TC
    body_ch = min(body_ch, n_ch)
    assert n_ch % body_ch == 0 and n_steps % TC == 0 and body_ch % 2 == 0
    n_iter = n_ch // body_ch
    pad_ch = n_ch + body_ch + 1
    xcols = pad_ch * CPC

    nc = bass.Bass()
    xt = nc.declare_dram_parameter("xt", [KD, xcols], BF16, isOutput=False)
    # combined weights: cols [0:512] = Wh (permuted), cols [512:1024] = [Wi; bh]
    # (rows 65:128 of the right half are zero padding); g blocks pre-scaled x2
    wcb = nc.declare_dram_parameter("wcb", [H, 2 * G4], BF16, isOutput=False)
    hout = nc.declare_dram_parameter("h_out", [H, BC], F32, isOutput=True)

    with tile.TileContext(nc) as tc:
        with (
            tc.tile_pool(name="weights", bufs=1) as wpool,
            tc.tile_pool(name="xin", bufs=1) as xpool,
            tc.tile_pool(name="state", bufs=1) as spool,
            tc.tile_pool(name="psum", bufs=1, space=bass.MemorySpace.PSUM) as ppool,
        ):
            w_sb = wpool.tile([H, 2 * G4], BF16, tag="w")
            wh_sb = w_sb[:, 0:G4]
            wi_sb = w_sb[:KD, G4 : 2 * G4]
            xs_all = xpool.tile([KD, body_ch * CPC], BF16, tag="xs")
            xs = [xs_all[:, k * CPC : (k + 1) * CPC] for k in range(body_ch)]
            # persistent state: st = sigmoid outputs [i|f|o|g'] (bf16),
            # cc = cell state (fp32), wk = [u|v] (fp32), tc_sb = tanh(c) (bf16)
            st = spool.tile([H, 4 * BC], BF16, tag="st")
            cc = spool.tile([H, BC], F32, tag="cc")
            wk = spool.tile([H, 2 * BC], F32, tag="wk")
            tc_sb = spool.tile([H, BC], BF16, tag="tc")
            h_sb = spool.tile([H, BC], BF16, tag="h")
            hf_sb = spool.tile([H, BC], F32, tag="hf")
            ps = [
                ppool.tile([H, TC * 128], F32, tag=f"ps{k}", name=f"ps{k}")
                for k in range(2)
            ]

            def xg_items(pi, xsrc):
                """The input-projection matmuls for one chunk into psum slot
                pi (each bank = one gate block; first write start=True zeroes
                it), as thunks. xg_split subdivides each bank's matmul."""
                items = []
                w = 512 // xg_split
                for gb in range(4):
                    for s in range(xg_split):

                        def mm(gb=gb, s=s):
                            nc.tensor.matmul(
                                ps[pi][
                                    :, gb * 512 + s * w : gb * 512 + (s + 1) * w
                                ],
                                wi_sb[:, gb * H : (gb + 1) * H],
                                xsrc[:, s * w : (s + 1) * w],
                                start=(s == 0),
                                stop=False,
                                skip_group_check=True,
                            )

                        items.append(mm)
                return items

            def xg_chunk(pi, xsrc):
                for it in xg_items(pi, xsrc):
                    it()

            def step(pi, j):
                """One LSTM timestep; z for step j lives at cols j*32 of each
                gate bank."""
                p = ps[pi]
                act = mybir.ActivationFunctionType
                alu = mybir.AluOpType
                for gb in range(4):
                    nc.tensor.matmul(
                        p[:, gb * 512 + j * BC : gb * 512 + (j + 1) * BC],
                        wh_sb[:, gb * H : (gb + 1) * H],
                        h_sb[:, :],
                        start=False,
                        stop=True,
                        skip_group_check=True,
                    )
                # strided view: the four gate blocks for step j sit 512 apart
                pz = p[:].rearrange("p (gb t z) -> p gb t z", gb=4, t=TC)
                # one sigmoid over [i|f|o|g'] (g weights pre-scaled x2 on host)
                nc.scalar.activation(
                    st[:].rearrange("p (a z) -> p a z", z=BC)[:, :, :],
                    pz[:, :, j, :],
                    act.Sigmoid,
                )
                # v = f * c  (reads previous c)
                nc.vector.tensor_mul(wk[:, BC : 2 * BC], st[:, BC : 2 * BC], cc[:, :])
                # u = (g' - 0.5) * i   [= 0.5 * i * tanh(zg)]
                nc.vector.scalar_tensor_tensor(
                    wk[:, 0:BC], st[:, 3 * BC : 4 * BC], 0.5, st[:, 0:BC],
                    alu.subtract, alu.mult,
                )
                # c = 2*u + v
                nc.vector.scalar_tensor_tensor(
                    cc[:, :], wk[:, 0:BC], 2.0, wk[:, BC : 2 * BC],
                    alu.mult, alu.add,
                )
                nc.scalar.activation(tc_sb[:, :], cc[:, :], act.Tanh)
                # h = o * tanh(c)
                nc.vector.tensor_mul(h_sb[:, :], st[:, 2 * BC : 3 * BC], tc_sb[:, :])

            def rec_chunk(pi, xg=()):
                """16 recurrence steps on slot pi; one xg matmul for the OTHER
                psum slot's next chunk is slipped in per 4 steps so the PE
                queue never sees a blocking burst at a chunk edge (tile-level
                WAR on the previous chunk's sigmoid reads keeps the bank
                zeroing correctly ordered)."""
                done = 0
                for j in range(TC):
                    step(pi, j)
                    want = (j + 1) * len(xg) // TC
                    while done < want:
                        xg[done]()
                        done += 1

            # ---- preamble ----
            nc.sync.dma_start(w_sb[:], wcb[:])
            nc.vector.memset(h_sb[:], 0.0)
            nc.vector.memset(cc[:], 0.0)
            nc.sync.dma_start(xs_all[:], xt[:, 0 : body_ch * CPC])
            xg_chunk(0, xs[0])

            # ---- main loop: body covers chunks B*i .. B*i+B-1; rec of chunk
            # n carries the xg matmuls of chunk n+1 (on the other psum slot);
            # the x slot consumed by next body's chunk m is refilled right
            # after its last reader (the xg carried by rec line m-1), a full
            # body ahead of its consumer ----
            with tc.For_i(
                0, n_iter, 1, hint_engines=(mybir.EngineType.PE,)
            ) as iv:
                base = iv * (body_ch * CPC)

                assert body_ch >= 8 and body_ch % 4 == 0
                for m in range(body_ch):
                    rec_chunk(
                        m % 2, xg_items((m + 1) % 2, xs[(m + 1) % body_ch])
                    )
                    # after line 4k+3, slots [4k..4k+3] have had their last
                    # read (lines 4k-1..4k+2); refill them with the chunks
                    # they'll serve next body (>= 4-line lead to consumers)
                    if m % 4 == 3:
                        g0 = m - 3
                        nc.sync.dma_start(
                            xs_all[:, g0 * CPC : (g0 + 4) * CPC],
                            xt[:, bass.ds(base + (body_ch + g0) * CPC, 4 * CPC)],
                        )

            # widen the final h to fp32 for the output DMA
            nc.vector.tensor_scalar_add(hf_sb[:, :], h_sb[:, :], 0.0)
            nc.sync.dma_start(hout[:], hf_sb[:])

    if strip_self:
        _strip_self_waits(nc)
    if transitive:
        _transitive_strip(nc)
    if legalize:  # CoreSim can't run the post-hoc clones; HW compile needs them
        _legalize_for_walrus(nc)
    return nc


def host_inputs(x, Wi, Wh, bh, n_steps=S, body_ch=BODY_CH):
    """Per-core input maps: transposed/padded x (bf16), permuted weights (bf16,
    g block pre-scaled by 2 for the tanh-via-sigmoid trick)."""
    n_ch = n_steps // TC
    body_ch = min(body_ch, n_ch)
    pad_ch = n_ch + body_ch + 1
    xcols = pad_ch * CPC
    gscale = np.ones((G4,), np.float32)
    gscale[384:512] = 2.0  # post-perm cols 384:512 are the g block
    wcb = np.zeros((H, 2 * G4), np.float32)
    wcb[:, 0:G4] = Wh[:, _PERM] * gscale
    wcb[0:D, G4:] = Wi[:, _PERM] * gscale
    wcb[D, G4:] = bh[_PERM] * gscale
    wcb = wcb.astype(NP_BF16)
    nb = x.shape[0] // NCORES
    in_maps = []
    for core in range(NCORES):
        xc = x[core * nb : (core + 1) * nb]  # [BC, n_steps, D]
        xtc = np.ascontiguousarray(xc.transpose(2, 1, 0)).reshape(D, n_steps * nb)
        full = np.zeros((KD, xcols), NP_BF16)
        full[:D, : n_steps * nb] = xtc.astype(NP_BF16)
        full[D, :] = 1.0
        in_maps.append({"xt": full, "wcb": wcb})
    return in_maps


_CACHE = {}


def _run(x, Wi, Wh, bh, trace=False):
    x = np.asarray(x, np.float32)
    if "nc" not in _CACHE:
        _CACHE["nc"] = build_bass()
    nc = _CACHE["nc"]
    in_maps = host_inputs(x, Wi, Wh, bh)
    res = run_bass_kernel_spmd(nc, in_maps, list(range(NCORES)), trace=trace)
    h_full = np.concatenate(
        [np.asarray(res.results[c]["h_out"]).astype(np.float32).T for c in range(NCORES)],
        axis=0,
    )  # [B, H]
    return h_full, res


def kernel(x, Wi, Wh, bh, Wo, bo):
    x = np.asarray(x, np.float32)
    Wi = np.asarray(Wi, np.float32)
    Wh = np.asarray(Wh, np.float32)
    bh = np.asarray(bh, np.float32)
    Wo = np.asarray(Wo, np.float32)
    bo = np.asarray(bo, np.float32)
    h_full, _ = _run(x, Wi, Wh, bh)
    return (h_full @ Wo + bo).astype(np.float32)
